# revision 5
# baseline (speedup 1.0000x reference)
"""Trainium2 Bass/Tile kernel: single-head attention (B=8, S=2048, E=1024, DQ=DV=128).

Data-parallel over the batch: one batch element per NeuronCore (8 cores), no
collectives. Host pre-transposes activations to [E, S] bf16 so the contraction
dim lands on SBUF partitions; everything else runs on-chip:

  qT/kT/vT = W.T @ xT          (PE, bf16 in / fp32 PSUM accum, bias added on DVE copy)
  v_aug    = transpose(vT) ++ ones column   (PE transpose; ones column makes the
                                             AV matmul emit softmax row sums for free)
  scoresT  = kT_chunk.T @ qT   ([keys, queries] layout; causal upper blocks skipped)
  attnT    = exp(scoresT/sqrt(DQ) + pad_bias)  (ACT; pad mask is a per-partition bias;
                                               no max-subtraction needed: |scores| < ~3)
  out[q,:] = (attnT.T @ v_aug)[:, :DV] * recip(row_sum)   (PE + DVE recip/scale)

Schedule (v3, pipelined stripes): q and k stream as paired [E, 512]-column
granules split across BOTH HWDGE rings (~420GB/s aggregate); after pair p the
kernel projects both granules and runs the scores column p for every ready key
block, so the serialized exp chain on the scalar engine starts at ~14us and
finishes mid-kernel instead of gating the tail.  v streams last in the same
stripe form; each v granule feeds v-proj, PE-transpose, and the AV chains for
its 4 query tiles.  Dummy matmuls on a junk tile pre-ramp the PE DVFS clock
(0.65->2.4GHz over ~3us) before the first real work.  The AV PSUM ring is
4 deep so the DVE normalize+store never stalls the matmul chains.
"""

import numpy as np
import ml_dtypes
from contextlib import ExitStack

B, S, E, DQ, DV = 8, 2048, 1024, 128, 128
EC = E // 128    # contraction chunks
SC = S // 128    # sequence chunks
QB = 512         # stripe / matmul moving-dim width
NG = S // QB     # stripe granules per tensor
RSQRT_DQ = 1.0 / float(np.sqrt(DQ))
NEG = np.float32(-1e9)
_BF16 = ml_dtypes.bfloat16

_prog = None


def _build_program():
    import concourse.bacc as bacc
    import concourse.mybir as mybir
    import concourse.tile as tile

    f32 = mybir.dt.float32
    bf16 = mybir.dt.bfloat16
    AF = mybir.ActivationFunctionType
    ALU = mybir.AluOpType

    nc = bacc.Bacc("TRN2", target_bir_lowering=False, debug=False)

    d_x = {n: nc.dram_tensor(n, [E, S], bf16, kind="ExternalInput").ap()
           for n in ("qT", "kT", "vT")}
    d_w = {n: nc.dram_tensor(n, [128, EC, 128], bf16, kind="ExternalInput").ap()
           for n in ("wq", "wk", "wv")}
    d_b = {n: nc.dram_tensor(n, [128, 1], f32, kind="ExternalInput").ap()
           for n in ("bq", "bk", "bv")}
    d_padb = nc.dram_tensor("padb", [128, SC], f32, kind="ExternalInput").ap()
    d_tri = nc.dram_tensor("tri", [128, 128], bf16, kind="ExternalInput").ap()
    d_eye = nc.dram_tensor("eye", [128, 128], bf16, kind="ExternalInput").ap()
    d_out = nc.dram_tensor("out", [S, DV], f32, kind="ExternalOutput").ap()

    with tile.TileContext(nc) as tc, ExitStack() as ctx:
        consts = ctx.enter_context(tc.tile_pool(name="consts", bufs=1))
        xin_p = ctx.enter_context(tc.tile_pool(name="xin", bufs=1))
        proj_p = ctx.enter_context(tc.tile_pool(name="proj", bufs=1))
        attn_p = ctx.enter_context(tc.tile_pool(name="attn", bufs=1))
        out_p = ctx.enter_context(tc.tile_pool(name="outp", bufs=4))
        # PSUM: proj ring 2 banks + scores ring 2 + AV/dummy ring 4 = 8
        ps_p = ctx.enter_context(tc.tile_pool(name="ps_p", bufs=2, space="PSUM"))
        ps_sc = ctx.enter_context(tc.tile_pool(name="ps_sc", bufs=2, space="PSUM"))
        ps_av = ctx.enter_context(tc.tile_pool(name="ps_av", bufs=4, space="PSUM"))

        # --- junk tile for PE clock-warmup matmuls (content irrelevant) ---
        junk = consts.tile([128, QB], bf16, tag="junk")
        nc.vector.memset(junk[:, :], 1.0)

        def warm_mm(n):
            # dummy matmuls: ramp/hold the PE DVFS clock during DMA waits.
            for _ in range(n):
                ps = ps_av.tile([128, QB], f32, tag="av", name="warmps")
                nc.tensor.matmul(ps[:, :], junk[:, 0:128], junk[:, :],
                                 start=True, stop=True)

        # One-time exp LUT load: scalar engine's first instruction.
        warm = consts.tile([128, 1], f32, tag="warm")
        nc.vector.memset(warm[:, :], 0.0)
        wo = consts.tile([128, 1], f32, tag="warmo")
        nc.scalar.activation(wo[:, :], warm[:, :], AF.Exp)

        # --- input stripes: granule g of tensor n = x[:, g*QB:(g+1)*QB],
        # [E, QB] = 8 chunk-slices [128, QB]; chunks 0-3 on sync, 4-7 scalar.
        xg = {"qT": [None] * NG, "kT": [None] * NG, "vT": [None] * NG}

        def stripe_dma(name, g):
            halves = []
            for h, eng in ((0, nc.sync), (1, nc.scalar)):
                t = xin_p.tile([128, 4, QB], bf16, tag=f"x{name[0]}{g}h{h}",
                               name=f"x{name[0]}{g}h{h}")
                src = d_x[name][h * 512:(h + 1) * 512, g * QB:(g + 1) * QB] \
                    .rearrange("(r p) s -> p r s", p=128)
                eng.dma_start(t[:, :, :], src)
                halves.append(t)
            xg[name][g] = halves

        def xchunk(name, g, c):
            return xg[name][g][c // 4][:, c % 4, :]

        w_sb = {}
        b_sb = {}

        def w_dma(eng, n):
            t = consts.tile([128, EC, 128], bf16, tag="w" + n, name="w" + n)
            eng.dma_start(t[:, :, :], d_w["w" + n])
            w_sb["w" + n] = t
            t = consts.tile([128, 1], f32, tag="b" + n, name="b" + n)
            eng.dma_start(t[:, :], d_b["b" + n])
            b_sb["b" + n] = t

        # consts first (small), then the (q,k) stripe pairs, then v stripes.
        w_dma(nc.sync, "q")
        w_dma(nc.scalar, "k")
        tri = consts.tile([128, 128], bf16, tag="tri")
        nc.sync.dma_start(tri[:, :], d_tri)
        padb = consts.tile([128, SC], f32, tag="padb")
        nc.scalar.dma_start(padb[:, :], d_padb)
        for g in range(NG):
            stripe_dma("qT", g)
            stripe_dma("kT", g)
        w_dma(nc.sync, "v")
        eye = consts.tile([128, 128], bf16, tag="eye")
        nc.scalar.dma_start(eye[:, :], d_eye)
        for g in range(NG):
            stripe_dma("vT", g)

        qT = proj_p.tile([128, S], bf16, tag="qT")
        kT = proj_p.tile([128, S], bf16, tag="kT")
        vT = proj_p.tile([128, S], bf16, tag="vT")

        def proj_stripe(name, g, dst):
            ps = ps_p.tile([128, QB], f32, tag="pp", name=f"pp_{name[0]}{g}")
            w = w_sb["w" + name[0]]
            for c in range(EC):
                nc.tensor.matmul(ps[:, :], w[:, c, :], xchunk(name, g, c),
                                 start=(c == 0), stop=(c == EC - 1))
            nc.vector.tensor_scalar(dst[:, g * QB:(g + 1) * QB], ps[:, :],
                                    b_sb["b" + name[0]][:, :], None, ALU.add)

        attnT = [attn_p.tile([128, S - j * 128], bf16, tag=f"attnT{j}",
                             name=f"attnT{j}")
                 for j in range(SC)]

        def scores_piece(j, p):
            # scoresT[j], query columns [p*QB, (p+1)*QB) -> exp -> attnT[j]
            q0 = max(p * QB, j * 128)
            m = (p + 1) * QB - q0
            ps = ps_sc.tile([128, QB], f32, tag="sc", name=f"sc{j}_{p}")
            nc.tensor.matmul(ps[:, 0:m], kT[:, j * 128:(j + 1) * 128],
                             qT[:, q0:q0 + m], start=True, stop=True)
            a0 = q0 - j * 128
            nc.scalar.activation(attnT[j][:, a0:a0 + m], ps[:, 0:m], AF.Exp,
                                 bias=padb[:, j:j + 1], scale=RSQRT_DQ)
            if p == j // 4:
                # in-block causal mask on the diagonal block (keep k <= q)
                nc.vector.tensor_mul(attnT[j][:, 0:128], attnT[j][:, 0:128],
                                     tri[:, :])

        # ---- pre-ramp the PE clock while the first stripes stream in ----
        warm_mm(12)

        # ---- (q,k) stripe pairs with the scores column for each pair ----
        for p in range(NG):
            proj_stripe("qT", p, qT)
            proj_stripe("kT", p, kT)
            for j in range(4 * p + 4):
                scores_piece(j, p)
            if p < 2:
                warm_mm(2)

        # ---- v stripes: proj + transpose(++ones) + AV chains per stripe ----
        vaug = []
        for g in range(NG):
            proj_stripe("vT", g, vT)
            for j in range(4 * g, 4 * g + 4):
                ps = ps_sc.tile([128, QB], bf16, tag="sc", name="vt")
                nc.tensor.transpose(ps[:, 0:128], vT[:, j * 128:(j + 1) * 128],
                                    eye[:, :])
                va = attn_p.tile([128, DV + 1], bf16, tag=f"vaug{j}")
                nc.vector.tensor_copy(va[:, 0:DV], ps[:, 0:128])
                nc.vector.memset(va[:, DV:DV + 1], 1.0)
                vaug.append(va)
            for i in range(4 * g, 4 * g + 4):
                ps = ps_av.tile([128, QB], f32, tag="av", name=f"av{i}")
                for j in range(i + 1):
                    nc.tensor.matmul(ps[:, 0:DV + 1],
                                     attnT[j][:, (i - j) * 128:(i - j) * 128 + 128],
                                     vaug[j][:, :], start=(j == 0), stop=(j == i))
                rec = out_p.tile([128, 1], f32, tag="rec")
                nc.vector.reciprocal(rec[:, :], ps[:, DV:DV + 1])
                ot = out_p.tile([128, DV], f32, tag="ot")
                nc.vector.tensor_scalar(ot[:, :], ps[:, 0:DV], rec[:, :], None,
                                        ALU.mult)
                eng = nc.sync if i % 2 == 0 else nc.scalar
                eng.dma_start(d_out[i * 128:(i + 1) * 128, :], ot[:, :])

    nc.compile()
    return nc


def _prep_inputs(pad_mask, query, key, value, Wq, bq, Wk, bk, Wv, bv):
    def wprep(w):
        return np.ascontiguousarray(
            np.asarray(w, np.float32).astype(_BF16).reshape(EC, 128, 128)
            .transpose(1, 0, 2))

    def bprep(v):
        return np.ascontiguousarray(np.asarray(v, np.float32).reshape(128, 1))

    shared = {
        "wq": wprep(Wq), "wk": wprep(Wk), "wv": wprep(Wv),
        "bq": bprep(bq), "bk": bprep(bk), "bv": bprep(bv),
        "tri": np.triu(np.ones((128, 128), np.float32)).astype(_BF16),
        "eye": np.eye(128, dtype=np.float32).astype(_BF16),
    }
    pad_mask = np.asarray(pad_mask)
    query = np.asarray(query, np.float32)
    key = np.asarray(key, np.float32)
    value = np.asarray(value, np.float32)
    in_maps = []
    for b in range(B):
        padb = np.ascontiguousarray(
            np.where(pad_mask[b], NEG, np.float32(0.0)).reshape(SC, 128).T)
        in_maps.append({
            **shared,
            "qT": query[b].T.astype(_BF16, order="C"),
            "kT": key[b].T.astype(_BF16, order="C"),
            "vT": value[b].T.astype(_BF16, order="C"),
            "padb": padb.astype(np.float32),
        })
    return in_maps


def _run(in_maps, trace=False, **kwargs):
    global _prog
    from concourse.bass_utils import run_bass_kernel_spmd
    if _prog is None:
        _prog = _build_program()
    return run_bass_kernel_spmd(_prog, in_maps, list(range(B)), trace=trace,
                                **kwargs)


def kernel(pad_mask, query, key, value, Wq, bq, Wk, bk, Wv, bv):
    in_maps = _prep_inputs(pad_mask, query, key, value, Wq, bq, Wk, bk, Wv, bv)
    res = _run(in_maps)
    out = np.stack([np.asarray(res.results[i]["out"]) for i in range(B)])
    return np.ascontiguousarray(out.astype(np.float32))


# revision 8
# speedup vs baseline: 1.0585x; 1.0585x over previous
"""Trainium2 Bass/Tile kernel: single-head attention (B=8, S=2048, E=1024, DQ=DV=128).

Data-parallel over the batch: one batch element per NeuronCore (8 cores), no
collectives. Host pre-transposes activations to [E, S] bf16 so the contraction
dim lands on SBUF partitions; everything else runs on-chip:

  qT/kT/vT = W.T @ xT          (PE, bf16 in / fp32 PSUM accum, bias added on DVE copy)
  v_aug    = transpose(vT) ++ ones column   (PE transpose; ones column makes the
                                             AV matmul emit softmax row sums for free)
  scoresT  = kT_chunk.T @ qT   ([keys, queries] layout; causal upper blocks skipped)
  attnT    = exp(scoresT/sqrt(DQ) + pad_bias)  (ACT; pad mask is a per-partition bias;
                                               no max-subtraction needed: |scores| < ~3)
  out[q,:] = (attnT.T @ v_aug)[:, :DV] * recip(row_sum)   (PE + DVE recip/scale)

Schedule (v3, pipelined stripes): q and k stream as paired [E, 512]-column
granules split across BOTH HWDGE rings (~420GB/s aggregate); after pair p the
kernel projects both granules and runs the scores column p for every ready key
block, so the serialized exp chain on the scalar engine starts at ~14us and
finishes mid-kernel instead of gating the tail.  v streams last in the same
stripe form; each v granule feeds v-proj, PE-transpose, and the AV chains for
its 4 query tiles.  Dummy matmuls on a junk tile pre-ramp the PE DVFS clock
(0.65->2.4GHz over ~3us) before the first real work.  The AV PSUM ring is
4 deep so the DVE normalize+store never stalls the matmul chains.
"""

import numpy as np
import ml_dtypes
from contextlib import ExitStack

B, S, E, DQ, DV = 8, 2048, 1024, 128, 128
EC = E // 128    # contraction chunks
SC = S // 128    # sequence chunks
QB = 512         # stripe / matmul moving-dim width
NG = S // QB     # stripe granules per tensor
RSQRT_DQ = 1.0 / float(np.sqrt(DQ))
NEG = np.float32(-1e9)
_BF16 = ml_dtypes.bfloat16

_prog = None


def _build_program():
    import concourse.bacc as bacc
    import concourse.mybir as mybir
    import concourse.tile as tile

    f32 = mybir.dt.float32
    bf16 = mybir.dt.bfloat16
    AF = mybir.ActivationFunctionType
    ALU = mybir.AluOpType

    nc = bacc.Bacc("TRN2", target_bir_lowering=False, debug=False)

    # stripe-blocked activations: [granule, partition, rowblock, col] so each
    # half-granule is one contiguous 512KB DMA with 4KB partition lines.
    d_x = {n + h: nc.dram_tensor(n + h, [NG, 128, 4, QB], bf16,
                                 kind="ExternalInput").ap()
           for n in ("qT", "kT", "vT") for h in ("S", "C")}
    d_w = {n: nc.dram_tensor(n, [128, EC, 128], bf16, kind="ExternalInput").ap()
           for n in ("wq", "wk", "wv")}
    d_b = {n: nc.dram_tensor(n, [128, 1], f32, kind="ExternalInput").ap()
           for n in ("bq", "bk", "bv")}
    d_padb = nc.dram_tensor("padb", [128, SC], f32, kind="ExternalInput").ap()
    d_tri = nc.dram_tensor("tri", [128, 128], bf16, kind="ExternalInput").ap()
    d_eye = nc.dram_tensor("eye", [128, 128], bf16, kind="ExternalInput").ap()
    d_out = nc.dram_tensor("out", [S, DV], f32, kind="ExternalOutput").ap()

    with tile.TileContext(nc) as tc, ExitStack() as ctx:
        consts = ctx.enter_context(tc.tile_pool(name="consts", bufs=1))
        xin_p = ctx.enter_context(tc.tile_pool(name="xin", bufs=1))
        proj_p = ctx.enter_context(tc.tile_pool(name="proj", bufs=1))
        attn_p = ctx.enter_context(tc.tile_pool(name="attn", bufs=1))
        out_p = ctx.enter_context(tc.tile_pool(name="outp", bufs=4))
        # PSUM: proj ring 2 banks + scores ring 2 + AV/dummy ring 4 = 8
        ps_p = ctx.enter_context(tc.tile_pool(name="ps_p", bufs=2, space="PSUM"))
        ps_sc = ctx.enter_context(tc.tile_pool(name="ps_sc", bufs=2, space="PSUM"))
        ps_av = ctx.enter_context(tc.tile_pool(name="ps_av", bufs=4, space="PSUM"))

        # --- junk tile for PE clock-warmup matmuls (content irrelevant) ---
        junk = consts.tile([128, QB], bf16, tag="junk")
        nc.vector.memset(junk[:, :], 1.0)

        def warm_mm(n):
            # dummy matmuls: ramp/hold the PE DVFS clock during DMA waits.
            for _ in range(n):
                ps = ps_av.tile([128, QB], f32, tag="av", name="warmps")
                nc.tensor.matmul(ps[:, :], junk[:, 0:128], junk[:, :],
                                 start=True, stop=True)

        # One-time exp LUT load: scalar engine's first instruction.
        warm = consts.tile([128, 1], f32, tag="warm")
        nc.vector.memset(warm[:, :], 0.0)
        wo = consts.tile([128, 1], f32, tag="warmo")
        nc.scalar.activation(wo[:, :], warm[:, :], AF.Exp)

        # --- input stripes: granule g of tensor n = x[:, g*QB:(g+1)*QB],
        # [E, QB] = 8 chunk-slices [128, QB]; chunks 0-3 on sync, 4-7 scalar.
        xg = {"qT": [None] * NG, "kT": [None] * NG, "vT": [None] * NG}

        def stripe_dma(name, g):
            halves = []
            for h, eng in (("S", nc.sync), ("C", nc.scalar)):
                t = xin_p.tile([128, 4, QB], bf16, tag=f"x{name[0]}{g}h{h}",
                               name=f"x{name[0]}{g}h{h}")
                eng.dma_start(t[:, :, :], d_x[name + h][g])
                halves.append(t)
            xg[name][g] = halves

        def xchunk(name, g, c):
            return xg[name][g][c // 4][:, c % 4, :]

        w_sb = {}
        b_sb = {}

        def w_dma(eng, n):
            t = consts.tile([128, EC, 128], bf16, tag="w" + n, name="w" + n)
            eng.dma_start(t[:, :, :], d_w["w" + n])
            w_sb["w" + n] = t
            t = consts.tile([128, 1], f32, tag="b" + n, name="b" + n)
            eng.dma_start(t[:, :], d_b["b" + n])
            b_sb["b" + n] = t

        # consts first (small), then the (q,k) stripe pairs, then v stripes.
        w_dma(nc.sync, "q")
        w_dma(nc.scalar, "k")
        tri = consts.tile([128, 128], bf16, tag="tri")
        nc.sync.dma_start(tri[:, :], d_tri)
        padb = consts.tile([128, SC], f32, tag="padb")
        nc.scalar.dma_start(padb[:, :], d_padb)
        for g in range(NG):
            stripe_dma("qT", g)
            stripe_dma("kT", g)
        w_dma(nc.sync, "v")
        eye = consts.tile([128, 128], bf16, tag="eye")
        nc.scalar.dma_start(eye[:, :], d_eye)
        for g in range(NG):
            stripe_dma("vT", g)

        qT = proj_p.tile([128, S], bf16, tag="qT")
        kT = proj_p.tile([128, S], bf16, tag="kT")
        vT = proj_p.tile([128, S], bf16, tag="vT")

        def proj_stripe(name, g, dst):
            ps = ps_p.tile([128, QB], f32, tag="pp", name=f"pp_{name[0]}{g}")
            w = w_sb["w" + name[0]]
            for c in range(EC):
                nc.tensor.matmul(ps[:, :], w[:, c, :], xchunk(name, g, c),
                                 start=(c == 0), stop=(c == EC - 1))
            nc.vector.tensor_scalar(dst[:, g * QB:(g + 1) * QB], ps[:, :],
                                    b_sb["b" + name[0]][:, :], None, ALU.add)

        attnT = [attn_p.tile([128, S - j * 128], bf16, tag=f"attnT{j}",
                             name=f"attnT{j}")
                 for j in range(SC)]

        def scores_piece(j, p):
            # scoresT[j], query columns [p*QB, (p+1)*QB) -> exp -> attnT[j]
            q0 = max(p * QB, j * 128)
            m = (p + 1) * QB - q0
            ps = ps_sc.tile([128, QB], f32, tag="sc", name=f"sc{j}_{p}")
            nc.tensor.matmul(ps[:, 0:m], kT[:, j * 128:(j + 1) * 128],
                             qT[:, q0:q0 + m], start=True, stop=True)
            a0 = q0 - j * 128
            nc.scalar.activation(attnT[j][:, a0:a0 + m], ps[:, 0:m], AF.Exp,
                                 bias=padb[:, j:j + 1], scale=RSQRT_DQ)
            if p == j // 4:
                # in-block causal mask on the diagonal block (keep k <= q)
                nc.vector.tensor_mul(attnT[j][:, 0:128], attnT[j][:, 0:128],
                                     tri[:, :])

        # ---- pre-ramp the PE clock while the first stripes stream in ----
        warm_mm(12)

        # ---- (q,k) stripe pairs with the scores column for each pair ----
        for p in range(NG):
            proj_stripe("qT", p, qT)
            proj_stripe("kT", p, kT)
            for j in range(4 * p + 4):
                scores_piece(j, p)
            if p < 2:
                warm_mm(2)

        # ---- v stripes: proj + transpose(++ones) + AV chains per stripe ----
        vaug = []
        for g in range(NG):
            proj_stripe("vT", g, vT)
            for j in range(4 * g, 4 * g + 4):
                ps = ps_sc.tile([128, QB], bf16, tag="sc", name="vt")
                nc.tensor.transpose(ps[:, 0:128], vT[:, j * 128:(j + 1) * 128],
                                    eye[:, :])
                va = attn_p.tile([128, DV + 1], bf16, tag=f"vaug{j}")
                nc.vector.tensor_copy(va[:, 0:DV], ps[:, 0:128])
                nc.vector.memset(va[:, DV:DV + 1], 1.0)
                vaug.append(va)
            for i in range(4 * g, 4 * g + 4):
                ps = ps_av.tile([128, QB], f32, tag="av", name=f"av{i}")
                for j in range(i + 1):
                    nc.tensor.matmul(ps[:, 0:DV + 1],
                                     attnT[j][:, (i - j) * 128:(i - j) * 128 + 128],
                                     vaug[j][:, :], start=(j == 0), stop=(j == i))
                rec = out_p.tile([128, 1], f32, tag="rec")
                nc.vector.reciprocal(rec[:, :], ps[:, DV:DV + 1])
                ot = out_p.tile([128, DV], f32, tag="ot")
                nc.vector.tensor_scalar(ot[:, :], ps[:, 0:DV], rec[:, :], None,
                                        ALU.mult)
                eng = nc.sync if i % 2 == 0 else nc.scalar
                eng.dma_start(d_out[i * 128:(i + 1) * 128, :], ot[:, :])

    nc.compile()
    return nc


def _prep_inputs(pad_mask, query, key, value, Wq, bq, Wk, bk, Wv, bv):
    def wprep(w):
        return np.ascontiguousarray(
            np.asarray(w, np.float32).astype(_BF16).reshape(EC, 128, 128)
            .transpose(1, 0, 2))

    def bprep(v):
        return np.ascontiguousarray(np.asarray(v, np.float32).reshape(128, 1))

    shared = {
        "wq": wprep(Wq), "wk": wprep(Wk), "wv": wprep(Wv),
        "bq": bprep(bq), "bk": bprep(bk), "bv": bprep(bv),
        "tri": np.triu(np.ones((128, 128), np.float32)).astype(_BF16),
        "eye": np.eye(128, dtype=np.float32).astype(_BF16),
    }
    pad_mask = np.asarray(pad_mask)
    query = np.asarray(query, np.float32)
    key = np.asarray(key, np.float32)
    value = np.asarray(value, np.float32)

    def xprep(x):
        # x [S, E] -> per-ring stripe blocks [NG, 128, 4, QB]:
        # [g, p, r, s] = x[g*QB+s, h*512 + r*128 + p]
        a = x.reshape(NG, QB, 2, 4, 128)
        return (np.ascontiguousarray(a[:, :, 0].transpose(0, 3, 2, 1)).astype(_BF16),
                np.ascontiguousarray(a[:, :, 1].transpose(0, 3, 2, 1)).astype(_BF16))

    in_maps = []
    for b in range(B):
        padb = np.ascontiguousarray(
            np.where(pad_mask[b], NEG, np.float32(0.0)).reshape(SC, 128).T)
        qS, qC = xprep(query[b])
        kS, kC = xprep(key[b])
        vS, vC = xprep(value[b])
        in_maps.append({
            **shared,
            "qTS": qS, "qTC": qC, "kTS": kS, "kTC": kC, "vTS": vS, "vTC": vC,
            "padb": padb.astype(np.float32),
        })
    return in_maps


def _run(in_maps, trace=False, **kwargs):
    global _prog
    from concourse.bass_utils import run_bass_kernel_spmd
    if _prog is None:
        _prog = _build_program()
    return run_bass_kernel_spmd(_prog, in_maps, list(range(B)), trace=trace,
                                **kwargs)


def kernel(pad_mask, query, key, value, Wq, bq, Wk, bk, Wv, bv):
    in_maps = _prep_inputs(pad_mask, query, key, value, Wq, bq, Wk, bk, Wv, bv)
    res = _run(in_maps)
    out = np.stack([np.asarray(res.results[i]["out"]) for i in range(B)])
    return np.ascontiguousarray(out.astype(np.float32))


# revision 14
# speedup vs baseline: 1.1646x; 1.1002x over previous
"""Trainium2 Bass/Tile kernel: single-head attention (B=8, S=2048, E=1024, DQ=DV=128).

Data-parallel over the batch: one batch element per NeuronCore (8 cores), no
collectives. Host pre-transposes activations into stripe-blocked bf16 so the
contraction dim lands on SBUF partitions and every DMA is a contiguous 512KB
block with 4KB partition lines; everything else runs on-chip:

  qT/kT/vT = W.T @ xT          (PE, bf16 in / fp32 PSUM accum, bias added on GPSIMD copy)
  v_aug    = transpose(vT) ++ ones column   (PE transpose; ones column makes the
                                             AV matmul emit softmax row sums for free)
  scoresT  = kT_chunk.T @ qT   ([keys, queries] layout; causal upper blocks skipped)
  attnT    = exp(scoresT/sqrt(DQ) + pad_bias)  (ACT; pad mask is a per-partition bias;
                                               no max-subtraction needed: |scores| < ~3)
  out[q,:] = (attnT.T @ v_aug)[:, :DV] * recip(row_sum)   (PE + DVE recip/scale)

Schedule (v4): q and k stream as paired [E, 512]-column granules split across
BOTH HWDGE rings (~420GB/s aggregate).  After pair p lands, the kernel projects
both granules and the scores column p runs for every ready key block, so the
serialized exp chain on the scalar engine starts at ~14us and ends mid-kernel
instead of gating the tail.  The scores pieces of each column are WOVEN between
the next pair's projection matmuls (and the first v projections for the last
column) so the exp-drain never idles the PE; dummy matmuls on a junk tile
pre-ramp and hold the PE DVFS clock (0.65->2.4GHz over ~3us of continuous
work).  Elementwise drains are split across engines: GPSIMD does the
projection-bias copies and v_aug packing, DVE does the causal tri mask and the
output normalize, keeping each engine off the matmul critical path.  Stripe
DMA issues are interspersed with the compute stream so neither ring's
descriptor queue ever blocks the scalar engine's exp work.
"""

import numpy as np
import ml_dtypes
from contextlib import ExitStack

B, S, E, DQ, DV = 8, 2048, 1024, 128, 128
EC = E // 128    # contraction chunks
SC = S // 128    # sequence chunks
QB = 512         # stripe / matmul moving-dim width
NG = S // QB     # stripe granules per tensor
RSQRT_DQ = 1.0 / float(np.sqrt(DQ))
NEG = np.float32(-1e9)
_BF16 = ml_dtypes.bfloat16

_prog = None


def _build_program():
    import concourse.bacc as bacc
    import concourse.mybir as mybir
    import concourse.tile as tile

    f32 = mybir.dt.float32
    bf16 = mybir.dt.bfloat16
    AF = mybir.ActivationFunctionType
    ALU = mybir.AluOpType

    nc = bacc.Bacc("TRN2", target_bir_lowering=False, debug=False)

    # stripe-blocked activations: [granule, partition, rowblock, col] so each
    # half-granule is one contiguous 512KB DMA with 4KB partition lines.
    d_x = {n + h: nc.dram_tensor(n + h, [NG, 128, 4, QB], bf16,
                                 kind="ExternalInput").ap()
           for n in ("qT", "kT", "vT") for h in ("S", "C")}
    d_w = {n: nc.dram_tensor(n, [128, EC, 128], bf16, kind="ExternalInput").ap()
           for n in ("wq", "wk", "wv")}
    d_b = {n: nc.dram_tensor(n, [128, 1], f32, kind="ExternalInput").ap()
           for n in ("bq", "bk", "bv")}
    d_padb = nc.dram_tensor("padb", [128, SC], f32, kind="ExternalInput").ap()
    d_tri = nc.dram_tensor("tri", [128, 128], bf16, kind="ExternalInput").ap()
    d_eye = nc.dram_tensor("eye", [128, 128], bf16, kind="ExternalInput").ap()
    d_out = nc.dram_tensor("out", [S, DV], f32, kind="ExternalOutput").ap()

    with tile.TileContext(nc) as tc, ExitStack() as ctx:
        consts = ctx.enter_context(tc.tile_pool(name="consts", bufs=1))
        xin_p = ctx.enter_context(tc.tile_pool(name="xin", bufs=1))
        proj_p = ctx.enter_context(tc.tile_pool(name="proj", bufs=1))
        attn_p = ctx.enter_context(tc.tile_pool(name="attn", bufs=1))
        out_p = ctx.enter_context(tc.tile_pool(name="outp", bufs=4))
        # PSUM: proj ring 2 banks + scores/vtrans ring 2 + AV/dummy ring 4 = 8
        ps_p = ctx.enter_context(tc.tile_pool(name="ps_p", bufs=2, space="PSUM"))
        ps_sc = ctx.enter_context(tc.tile_pool(name="ps_sc", bufs=2, space="PSUM"))
        ps_av = ctx.enter_context(tc.tile_pool(name="ps_av", bufs=4, space="PSUM"))

        # --- junk tile for PE clock-warmup matmuls (content irrelevant) ---
        junk = consts.tile([128, QB], bf16, tag="junk")
        nc.vector.memset(junk[:, :], 1.0)

        def warm_mm(n):
            # dummy matmuls: ramp/hold the PE DVFS clock during DMA waits.
            for _ in range(n):
                ps = ps_av.tile([128, QB], f32, tag="av", name="warmps")
                nc.tensor.matmul(ps[:, :], junk[:, 0:128], junk[:, :],
                                 start=True, stop=True)

        # One-time exp LUT load: scalar engine's first instruction.
        warm = consts.tile([128, 1], f32, tag="warm")
        nc.vector.memset(warm[:, :], 0.0)
        wo = consts.tile([128, 1], f32, tag="warmo")
        nc.scalar.activation(wo[:, :], warm[:, :], AF.Exp)

        # --- input stripes: granule g of tensor n, chunks 0-3 sync, 4-7 scalar
        xg = {"qT": [None] * NG, "kT": [None] * NG, "vT": [None] * NG}

        def stripe_dma(name, g):
            halves = []
            for h, eng in (("S", nc.sync), ("C", nc.scalar)):
                t = xin_p.tile([128, 4, QB], bf16, tag=f"x{name[0]}{g}h{h}",
                               name=f"x{name[0]}{g}h{h}")
                eng.dma_start(t[:, :, :], d_x[name + h][g])
                halves.append(t)
            xg[name][g] = halves

        def xchunk(name, g, c):
            return xg[name][g][c // 4][:, c % 4, :]

        w_sb = {}
        b_sb = {}

        def w_dma(eng, n):
            t = consts.tile([128, EC, 128], bf16, tag="w" + n, name="w" + n)
            eng.dma_start(t[:, :, :], d_w["w" + n])
            w_sb["w" + n] = t
            t = consts.tile([128, 1], f32, tag="b" + n, name="b" + n)
            eng.dma_start(t[:, :], d_b["b" + n])
            b_sb["b" + n] = t

        # consts + first two (q,k) pairs up front; later stripes are issued
        # mid-stream so the scalar engine's exp work is never queued behind a
        # full DMA descriptor ring.
        w_dma(nc.sync, "q")
        w_dma(nc.scalar, "k")
        tri = consts.tile([128, 128], bf16, tag="tri")
        nc.sync.dma_start(tri[:, :], d_tri)
        padb = consts.tile([128, SC], f32, tag="padb")
        nc.scalar.dma_start(padb[:, :], d_padb)
        stripe_dma("qT", 0)
        stripe_dma("kT", 0)
        stripe_dma("qT", 1)
        stripe_dma("kT", 1)

        qT = proj_p.tile([128, S], bf16, tag="qT")
        kT = proj_p.tile([128, S], bf16, tag="kT")
        vT = proj_p.tile([128, S], bf16, tag="vT")

        def proj_mms(name, g, dst):
            # returns (per-matmul emitters, bias-drain emitter)
            ps = ps_p.tile([128, QB], f32, tag="pp", name=f"pp_{name[0]}{g}")
            w = w_sb["w" + name[0]]

            def mm(c, ps=ps, w=w, name=name, g=g):
                nc.tensor.matmul(ps[:, :], w[:, c, :], xchunk(name, g, c),
                                 start=(c == 0), stop=(c == EC - 1))

            def drain(ps=ps, name=name, g=g, dst=dst):
                nc.vector.tensor_scalar(dst[:, g * QB:(g + 1) * QB], ps[:, :],
                                        b_sb["b" + name[0]][:, :], None, ALU.add)

            return [lambda c=c: mm(c) for c in range(EC)], drain

        attnT = [attn_p.tile([128, S - j * 128], bf16, tag=f"attnT{j}",
                             name=f"attnT{j}")
                 for j in range(SC)]

        def scores_piece(j, p):
            # scoresT[j], query columns [p*QB, (p+1)*QB) -> exp -> attnT[j]
            q0 = max(p * QB, j * 128)
            m = (p + 1) * QB - q0
            ps = ps_sc.tile([128, QB], f32, tag="sc", name=f"sc{j}_{p}")
            nc.tensor.matmul(ps[:, 0:m], kT[:, j * 128:(j + 1) * 128],
                             qT[:, q0:q0 + m], start=True, stop=True)
            a0 = q0 - j * 128
            nc.scalar.activation(attnT[j][:, a0:a0 + m], ps[:, 0:m], AF.Exp,
                                 bias=padb[:, j:j + 1], scale=RSQRT_DQ)
            if p == j // 4:
                # in-block causal mask on the diagonal block (keep k <= q);
                # on GPSIMD (all-SBUF) to keep DVE free for the PSUM drains
                nc.gpsimd.tensor_mul(attnT[j][:, 0:128], attnT[j][:, 0:128],
                                     tri[:, :])

        def weave(pieces, fillers, extra_dummies=0):
            # emit scores pieces with filler matmuls spread between them so
            # the exp drain (one piece / ~560ns) never starves the PE.
            na, nb = len(pieces), len(fillers)
            bi, acc = 0, 0.0
            r = nb / max(na, 1)
            for a in pieces:
                a()
                acc += r
                while acc >= 1.0 and bi < nb:
                    fillers[bi]()
                    bi += 1
                    acc -= 1.0
                if extra_dummies:
                    warm_mm(1)
                    extra_dummies -= 1
            while bi < nb:
                fillers[bi]()
                bi += 1
            warm_mm(extra_dummies)

        # ---- pre-ramp the PE clock while the first stripes stream in ----
        warm_mm(12)

        # ---- pair 0 projections ----
        q_mms, q_drain = proj_mms("qT", 0, qT)
        k_mms, k_drain = proj_mms("kT", 0, kT)
        for f in q_mms:
            f()
        q_drain()
        for f in k_mms:
            f()
        k_drain()

        # ---- stripe pairs p=1..3 woven with the scores column p-1 ----
        for p in range(NG):
            if p + 2 < NG:
                stripe_dma("qT", p + 2)
                stripe_dma("kT", p + 2)
            elif p + 2 == NG:
                w_dma(nc.sync, "v")
                eye = consts.tile([128, 128], bf16, tag="eye")
                nc.scalar.dma_start(eye[:, :], d_eye)
                stripe_dma("vT", 0)
                stripe_dma("vT", 1)
            else:
                stripe_dma("vT", 2)
                stripe_dma("vT", 3)
            pieces = [lambda j=j, p=p: scores_piece(j, p) for j in range(4 * p + 4)]
            if p + 1 < NG:
                q_mms, q_drain = proj_mms("qT", p + 1, qT)
                k_mms, k_drain = proj_mms("kT", p + 1, kT)
                half = len(pieces) // 2
                weave(pieces[:half], q_mms, extra_dummies=(8 if p == 0 else 0))
                q_drain()
                weave(pieces[half:], k_mms, extra_dummies=(4 if p == 0 else 0))
                k_drain()
            else:
                # last column: weave with the first two v projections
                v_mms0, v_drain0 = proj_mms("vT", 0, vT)
                v_mms1, v_drain1 = proj_mms("vT", 1, vT)
                half = len(pieces) // 2
                weave(pieces[:half], v_mms0)
                v_drain0()
                weave(pieces[half:], v_mms1)
                v_drain1()

        # ---- v stripes: transpose(++ones) + AV chains per stripe ----
        vaug = [None] * SC
        for g in range(NG):
            if g + 2 < NG:
                v_mms, v_drain = proj_mms("vT", g + 2, vT)
                for f in v_mms:
                    f()
                v_drain()
            for j in range(4 * g, 4 * g + 4):
                ps = ps_sc.tile([128, QB], bf16, tag="sc", name="vt")
                nc.tensor.transpose(ps[:, 0:128], vT[:, j * 128:(j + 1) * 128],
                                    eye[:, :])
                va = attn_p.tile([128, DV + 1], bf16, tag=f"vaug{j}")
                # PSUM->SBUF copy on the scalar engine (idle once exps finish)
                nc.scalar.activation(va[:, 0:DV], ps[:, 0:128], AF.Copy)
                nc.gpsimd.memset(va[:, DV:DV + 1], 1.0)
                vaug[j] = va
            for i in range(4 * g, 4 * g + 4):
                ps = ps_av.tile([128, QB], f32, tag="av", name=f"av{i}")
                for j in range(i + 1):
                    nc.tensor.matmul(ps[:, 0:DV + 1],
                                     attnT[j][:, (i - j) * 128:(i - j) * 128 + 128],
                                     vaug[j][:, :],
                                     start=(j == 0), stop=(j == i))
                rec = out_p.tile([128, 1], f32, tag="rec")
                nc.vector.reciprocal(rec[:, :], ps[:, DV:DV + 1])
                ot = out_p.tile([128, DV], f32, tag="ot")
                nc.vector.tensor_scalar(ot[:, :], ps[:, 0:DV], rec[:, :], None,
                                        ALU.mult)
                eng = nc.sync if i % 2 == 0 else nc.scalar
                eng.dma_start(d_out[i * 128:(i + 1) * 128, :], ot[:, :])

    nc.compile()
    return nc


def _prep_inputs(pad_mask, query, key, value, Wq, bq, Wk, bk, Wv, bv):
    def wprep(w):
        return np.ascontiguousarray(
            np.asarray(w, np.float32).astype(_BF16).reshape(EC, 128, 128)
            .transpose(1, 0, 2))

    def bprep(v):
        return np.ascontiguousarray(np.asarray(v, np.float32).reshape(128, 1))

    shared = {
        "wq": wprep(Wq), "wk": wprep(Wk), "wv": wprep(Wv),
        "bq": bprep(bq), "bk": bprep(bk), "bv": bprep(bv),
        "tri": np.triu(np.ones((128, 128), np.float32)).astype(_BF16),
        "eye": np.eye(128, dtype=np.float32).astype(_BF16),
    }
    pad_mask = np.asarray(pad_mask)
    query = np.asarray(query, np.float32)
    key = np.asarray(key, np.float32)
    value = np.asarray(value, np.float32)

    def xprep(x):
        # x [S, E] -> per-ring stripe blocks [NG, 128, 4, QB]:
        # [g, p, r, s] = x[g*QB+s, h*512 + r*128 + p]
        a = x.reshape(NG, QB, 2, 4, 128)
        return (np.ascontiguousarray(a[:, :, 0].transpose(0, 3, 2, 1)).astype(_BF16),
                np.ascontiguousarray(a[:, :, 1].transpose(0, 3, 2, 1)).astype(_BF16))

    in_maps = []
    for b in range(B):
        padb = np.ascontiguousarray(
            np.where(pad_mask[b], NEG, np.float32(0.0)).reshape(SC, 128).T)
        qS, qC = xprep(query[b])
        kS, kC = xprep(key[b])
        vS, vC = xprep(value[b])
        in_maps.append({
            **shared,
            "qTS": qS, "qTC": qC, "kTS": kS, "kTC": kC, "vTS": vS, "vTC": vC,
            "padb": padb.astype(np.float32),
        })
    return in_maps


def _run(in_maps, trace=False, **kwargs):
    global _prog
    from concourse.bass_utils import run_bass_kernel_spmd
    if _prog is None:
        _prog = _build_program()
    return run_bass_kernel_spmd(_prog, in_maps, list(range(B)), trace=trace,
                                **kwargs)


def kernel(pad_mask, query, key, value, Wq, bq, Wk, bk, Wv, bv):
    in_maps = _prep_inputs(pad_mask, query, key, value, Wq, bq, Wk, bk, Wv, bv)
    res = _run(in_maps)
    out = np.stack([np.asarray(res.results[i]["out"]) for i in range(B)])
    return np.ascontiguousarray(out.astype(np.float32))
